# revision 17
# baseline (speedup 1.0000x reference)
"""Trainium2 Bass kernel for ChebConv(K=2) + MLP net (nn_Net_63299228009240).

Data-parallel over 8 NeuronCores: batch 4096 -> 512 per core.

Math (per sample, reordered to cut SpMM flops):
    u = x @ w0                [200, 32]
    v = x @ w1                [200, 32]
    t1 = a @ v                [200, 32]   ((a @ x) @ w1 == a @ (x @ w1))
    h = elu(u + t1 + b_conv)  -> flat [6400]
    y1 = relu(h @ wd1 + bd1); y2 = relu(y1 @ wd2 + bd2)
    y3 = relu(y2 @ wd3 + bd3); out = sigmoid(y3 @ wd4 + bd4)

elu(s) = relu(s) + exp(min(s, 0)) - 1; the -1 is folded into bd1 on host
(G = elu + 1; (G-1) @ wd1 + bd1 = G @ wd1 + (bd1 - colsum(wd1))).

Device layout: activations live "feature-on-partition, batch-on-free".
x is pre-transposed on host per core into XT[(n,f), b] node-pair tiles.
The SpMM needs node-on-partition, so V is shuffled [(n,c),b] -> [n,(c,b)]
via SBUF->SBUF DMAs, and T1 shuffled back after the a-matmul.
"""

import sys

sys.path.insert(0, "/opt/trn_rl_repo")

from contextlib import ExitStack

import numpy as np
import ml_dtypes

import concourse.mybir as mybir
import concourse.tile as tile
from concourse import bacc
from concourse.alu_op_type import AluOpType
from concourse.bass_utils import run_bass_kernel_spmd

BF16 = mybir.dt.bfloat16
F32 = mybir.dt.float32
NPBF16 = ml_dtypes.bfloat16

NCORES = 8
B, NN, F, C = 4096, 200, 64, 32
H1, H2, H3 = 512, 256, 128
BLOC = B // NCORES  # 512 samples per core
NB = 256            # batch chunk on device
NCH = BLOC // NB    # 2 chunks
NT = NN // 2        # 100 node-pair tiles [(2 nodes x 64 f) = 128, NB]
KT = NN // 4        # 50 H k-tiles [(4 nodes x 32 c) = 128, NB]
XBLK = 20           # node-pairs per XT load DMA
MT1 = 100           # SpMM output tile rows (n_out per tile)

_CACHE = {}


def _emit(nc, tc, ctx, d):
    singles = ctx.enter_context(tc.tile_pool(name="singles", bufs=1))
    xtp = ctx.enter_context(tc.tile_pool(name="xtp", bufs=2))
    upool = ctx.enter_context(tc.tile_pool(name="upool", bufs=1))
    t1kp = ctx.enter_context(tc.tile_pool(name="t1kp", bufs=1))
    vpool = ctx.enter_context(tc.tile_pool(name="vpool", bufs=1))
    vstp = ctx.enter_context(tc.tile_pool(name="vstp", bufs=4))
    t1stp = ctx.enter_context(tc.tile_pool(name="t1stp", bufs=4))
    sp = ctx.enter_context(tc.tile_pool(name="sp", bufs=4))
    mnp = ctx.enter_context(tc.tile_pool(name="mnp", bufs=4))
    ep = ctx.enter_context(tc.tile_pool(name="ep", bufs=4))
    gp = ctx.enter_context(tc.tile_pool(name="gp", bufs=8))
    yp = ctx.enter_context(tc.tile_pool(name="yp", bufs=2))
    psA = ctx.enter_context(tc.tile_pool(name="psA", bufs=3, space="PSUM"))
    psB = ctx.enter_context(tc.tile_pool(name="psB", bufs=2, space="PSUM"))
    psC = ctx.enter_context(tc.tile_pool(name="psC", bufs=2, space="PSUM"))
    psD = ctx.enter_context(tc.tile_pool(name="psD", bufs=1, space="PSUM"))

    # ---- load weights/constants once ----
    wua = singles.tile([128, 128], BF16)
    wub = singles.tile([128, 128], BF16)
    wva = singles.tile([128, 128], BF16)
    wvb = singles.tile([128, 128], BF16)
    atp = singles.tile([128, 2, NN], BF16)
    wd1 = singles.tile([128, KT, H1], BF16)
    wd2 = singles.tile([128, 4, H2], BF16)
    wd3 = singles.tile([128, 2, H3], BF16)
    wd4 = singles.tile([128, 1], BF16)
    bcv = singles.tile([128, 1], F32)
    bd1 = singles.tile([128, 4], F32)
    bd2 = singles.tile([128, 2], F32)
    bd3 = singles.tile([128, 1], F32)
    bd4 = singles.tile([1, 1], F32)
    for sb, nm in [(wua, "wua"), (wub, "wub"), (wva, "wva"), (wvb, "wvb"),
                   (atp, "atp"), (wd1, "wd1s"), (wd2, "wd2s"), (wd3, "wd3s"),
                   (wd4, "wd4s"), (bcv, "bconv"), (bd1, "bd1"), (bd2, "bd2"),
                   (bd3, "bd3"), (bd4, "bd4")]:
        nc.sync.dma_start(out=sb[:], in_=d[nm][:])

    for ch in range(NCH):
        # ---- XT loads (double-buffered blocks of XBLK node-pairs) ----
        xtb = []
        for xb in range(NT // XBLK):
            t = xtp.tile([128, XBLK, NB], BF16, tag="xt")
            nc.sync.dma_start(
                out=t[:], in_=d["xt"][:, ch, xb * XBLK:(xb + 1) * XBLK, :])
            xtb.append(t)

        def xtile(t):
            return xtb[t // XBLK][:, t % XBLK, :]

        u_sb = upool.tile([128, KT, NB], BF16, tag="u")
        t1k = t1kp.tile([128, KT, NB], BF16, tag="t1k")
        v0 = vpool.tile([128, 32, NB], BF16, tag="v0")
        v1 = vpool.tile([128, 32, NB], BF16, tag="v1")

        # ---- conv matmuls + evac + V shuffle ----
        for pt in range(KT):  # psum tile pt covers nodes 4pt..4pt+3
            t0, t1_ = 2 * pt, 2 * pt + 1
            up = psA.tile([128, NB], F32, tag="psA")
            nc.tensor.matmul(up[:], wua[:], xtile(t0), start=True, stop=False)
            nc.tensor.matmul(up[:], wub[:], xtile(t1_), start=False, stop=True)
            # U evac with conv bias folded in (k-tile layout [(n%4,c), b])
            nc.vector.tensor_scalar_add(u_sb[:, pt, :], up[:], bcv[:])

            vps = psA.tile([128, NB], F32, tag="psA")
            nc.tensor.matmul(vps[:], wva[:], xtile(t0), start=True, stop=False)
            nc.tensor.matmul(vps[:], wvb[:], xtile(t1_), start=False, stop=True)
            vst = vstp.tile([128, NB], BF16, tag="vst")
            nc.scalar.copy(out=vst[:], in_=vps[:])
            # V shuffle: [(n%4)*32+c, b] -> V[4pt+q, c, b]; src partition walk
            # (q, c) is monotone so one DMA moves all 4 nodes.
            if pt < 32:
                dst = v0[4 * pt:4 * pt + 4, :, :]
            else:
                dst = v1[4 * (pt - 32):4 * (pt - 32) + 4, :, :]
            nc.sync.dma_start(out=dst, in_=vst[:])

        # ---- SpMM: T1[n_out, b] = aT.T @ V, one psum tile per (mt, c) ----
        # at columns are host-permuted r-major (m = r*25 + q for n_out =
        # mt*100 + 4q + r), so each psum tile's partition walk matches
        # t1k[r*32+c, 25mt+q, b] directly: one SBUF->SBUF DMA per tile.
        t1kv = t1k.rearrange("(r c) k b -> c r k b", r=4)
        for mt in range(2):
            lhs0 = atp[:, 0, mt * MT1:(mt + 1) * MT1]
            lhs1 = atp[0:72, 1, mt * MT1:(mt + 1) * MT1]
            for c in range(C):
                bp = psB.tile([MT1, NB], F32, tag="psB")
                nc.tensor.matmul(bp[:], lhs0, v0[:, c, :], start=True,
                                 stop=False)
                nc.tensor.matmul(bp[:], lhs1, v1[0:72, c, :], start=False,
                                 stop=True)
                t1s = t1stp.tile([MT1, NB], BF16, tag="t1st")
                nc.vector.tensor_copy(t1s[:], bp[:])
                nc.scalar.dma_start(
                    out=t1kv[c, :, 25 * mt:25 * (mt + 1), :], in_=t1s[:])

        # ---- ELU -> G, and d1 matmuls (kt-outer, 4 open psum groups) ----
        d1c = [psC.tile([128, 2, NB], F32, tag="psC", name=f"d1c{ch}_{j}")
               for j in range(2)]
        for kt in range(KT):
            s = sp.tile([128, NB], BF16, tag="s")
            nc.vector.tensor_add(s[:], u_sb[:, kt, :], t1k[:, kt, :])
            mn = mnp.tile([128, NB], BF16, tag="mn")
            nc.vector.tensor_scalar_min(mn[:], s[:], 0.0)
            e = ep.tile([128, NB], BF16, tag="e")
            nc.scalar.activation(e[:], mn[:], mybir.ActivationFunctionType.Exp)
            g = gp.tile([128, NB], BF16, tag="g")
            nc.vector.scalar_tensor_tensor(
                out=g[:], in0=s[:], scalar=0.0, in1=e[:],
                op0=AluOpType.max, op1=AluOpType.add)
            for mt in range(4):
                nc.tensor.matmul(
                    d1c[mt // 2][:, mt % 2, :],
                    wd1[:, kt, mt * 128:(mt + 1) * 128], g[:],
                    start=(kt == 0), stop=(kt == KT - 1))

        y1 = yp.tile([128, 4, NB], BF16, tag="y1")
        for mt in range(4):
            nc.scalar.activation(
                y1[:, mt, :], d1c[mt // 2][:, mt % 2, :],
                mybir.ActivationFunctionType.Relu, bias=bd1[:, mt:mt + 1])

        # ---- d2 ----
        y2 = yp.tile([128, 2, NB], BF16, tag="y2")
        for mt in range(2):
            dp = psD.tile([128, NB], F32, tag="psD")
            for kt in range(4):
                nc.tensor.matmul(
                    dp[:], wd2[:, kt, mt * 128:(mt + 1) * 128], y1[:, kt, :],
                    start=(kt == 0), stop=(kt == 3))
            nc.scalar.activation(
                y2[:, mt, :], dp[:],
                mybir.ActivationFunctionType.Relu, bias=bd2[:, mt:mt + 1])

        # ---- d3 ----
        y3 = yp.tile([128, NB], BF16, tag="y3")
        dp3 = psD.tile([128, NB], F32, tag="psD")
        for kt in range(2):
            nc.tensor.matmul(dp3[:], wd3[:, kt, :], y2[:, kt, :],
                             start=(kt == 0), stop=(kt == 1))
        nc.scalar.activation(y3[:], dp3[:],
                             mybir.ActivationFunctionType.Relu, bias=bd3[:])

        # ---- d4 + sigmoid ----
        dp4 = psD.tile([1, NB], F32, tag="psD")
        nc.tensor.matmul(dp4[:], wd4[:], y3[:], start=True, stop=True)
        yo = yp.tile([1, NB], F32, tag="yo")
        nc.scalar.activation(yo[:], dp4[:],
                             mybir.ActivationFunctionType.Sigmoid,
                             bias=bd4[:])
        nc.sync.dma_start(out=d["y"][0:1, ch * NB:(ch + 1) * NB], in_=yo[:])


def _build():
    if "nc" in _CACHE:
        return _CACHE["nc"]
    nc = bacc.Bacc("TRN2", target_bir_lowering=False, debug=False,
                   num_devices=NCORES)
    d = {
        "xt": nc.dram_tensor("xt", [128, NCH, NT, NB], BF16, kind="ExternalInput"),
        "wua": nc.dram_tensor("wua", [128, 128], BF16, kind="ExternalInput"),
        "wub": nc.dram_tensor("wub", [128, 128], BF16, kind="ExternalInput"),
        "wva": nc.dram_tensor("wva", [128, 128], BF16, kind="ExternalInput"),
        "wvb": nc.dram_tensor("wvb", [128, 128], BF16, kind="ExternalInput"),
        "atp": nc.dram_tensor("atp", [128, 2, NN], BF16, kind="ExternalInput"),
        "wd1s": nc.dram_tensor("wd1s", [128, KT, H1], BF16, kind="ExternalInput"),
        "wd2s": nc.dram_tensor("wd2s", [128, 4, H2], BF16, kind="ExternalInput"),
        "wd3s": nc.dram_tensor("wd3s", [128, 2, H3], BF16, kind="ExternalInput"),
        "wd4s": nc.dram_tensor("wd4s", [128, 1], BF16, kind="ExternalInput"),
        "bconv": nc.dram_tensor("bconv", [128, 1], F32, kind="ExternalInput"),
        "bd1": nc.dram_tensor("bd1", [128, 4], F32, kind="ExternalInput"),
        "bd2": nc.dram_tensor("bd2", [128, 2], F32, kind="ExternalInput"),
        "bd3": nc.dram_tensor("bd3", [128, 1], F32, kind="ExternalInput"),
        "bd4": nc.dram_tensor("bd4", [1, 1], F32, kind="ExternalInput"),
        "y": nc.dram_tensor("y", [1, BLOC], F32, kind="ExternalOutput"),
    }
    with tile.TileContext(nc) as tc, ExitStack() as ctx:
        _emit(nc, tc, ctx, d)
    nc.compile()
    _CACHE["nc"] = nc
    return nc


def _host_prep(x, a, w0, w1, b_conv, w_d1, b_d1, w_d2, b_d2, w_d3, b_d3,
               w_d4, b_d4):
    """Build per-core in_maps (shared weight arrays built once)."""
    w0 = np.asarray(w0, np.float32)
    w1 = np.asarray(w1, np.float32)
    # block-diag conv weights: lhsT[(nl*64+f), m]; A-variant fills out cols
    # [0:64) (nodes 4pt,4pt+1), B-variant cols [64:128) (nodes 4pt+2,4pt+3)
    wua = np.zeros((128, 128), np.float32)
    wub = np.zeros((128, 128), np.float32)
    wva = np.zeros((128, 128), np.float32)
    wvb = np.zeros((128, 128), np.float32)
    for nl in range(2):
        r = slice(nl * 64, nl * 64 + 64)
        wua[r, nl * 32:nl * 32 + 32] = w0
        wub[r, 64 + nl * 32:64 + nl * 32 + 32] = w0
        wva[r, nl * 32:nl * 32 + 32] = w1
        wvb[r, 64 + nl * 32:64 + nl * 32 + 32] = w1

    # permute at columns r-major within each mt block: col mt*100+r*25+q
    # holds n_out = mt*100+4q+r, so SpMM psum partitions land t1k-ready
    perm = np.empty(NN, np.int64)
    for mt in range(2):
        for q in range(25):
            for r in range(4):
                perm[mt * 100 + r * 25 + q] = mt * 100 + 4 * q + r
    atp = np.zeros((128, 2, NN), np.float32)
    at = np.asarray(a, np.float32).T[:, perm]  # at[n, m]
    atp[:, 0, :] = at[0:128, :]
    atp[0:72, 1, :] = at[128:200, :]

    w_d1 = np.asarray(w_d1, np.float32)
    wd1s = np.ascontiguousarray(w_d1.reshape(KT, 128, H1).transpose(1, 0, 2))
    wd2s = np.ascontiguousarray(
        np.asarray(w_d2, np.float32).reshape(4, 128, H2).transpose(1, 0, 2))
    wd3s = np.ascontiguousarray(
        np.asarray(w_d3, np.float32).reshape(2, 128, H3).transpose(1, 0, 2))
    wd4s = np.asarray(w_d4, np.float32).reshape(128, 1)

    p = np.arange(128)
    bconv_v = np.asarray(b_conv, np.float32)[p % 32].reshape(128, 1)
    bd1c = np.asarray(b_d1, np.float32) - w_d1.sum(axis=0)  # fold elu's -1
    bd1_v = np.ascontiguousarray(bd1c.reshape(4, 128).T)
    bd2_v = np.ascontiguousarray(
        np.asarray(b_d2, np.float32).reshape(2, 128).T)
    bd3_v = np.asarray(b_d3, np.float32).reshape(128, 1)
    bd4_v = np.asarray(b_d4, np.float32).reshape(1, 1)

    shared = {
        "wua": wua.astype(NPBF16), "wub": wub.astype(NPBF16),
        "wva": wva.astype(NPBF16), "wvb": wvb.astype(NPBF16),
        "atp": atp.astype(NPBF16), "wd1s": wd1s.astype(NPBF16),
        "wd2s": wd2s.astype(NPBF16), "wd3s": wd3s.astype(NPBF16),
        "wd4s": wd4s.astype(NPBF16),
        "bconv": np.ascontiguousarray(bconv_v, np.float32),
        "bd1": bd1_v.astype(np.float32), "bd2": bd2_v.astype(np.float32),
        "bd3": bd3_v, "bd4": bd4_v,
    }

    x = np.asarray(x, np.float32)
    in_maps = []
    for i in range(NCORES):
        xs = x[i * BLOC:(i + 1) * BLOC]                    # [512, 200, 64]
        xtf = xs.transpose(1, 2, 0).reshape(NN * F, BLOC)  # [(n,f), b]
        xti = np.ascontiguousarray(
            xtf.reshape(NT, 128, NCH, NB).transpose(1, 2, 0, 3)).astype(NPBF16)
        in_maps.append({"xt": xti, **shared})
    return in_maps


def kernel(**inputs):
    nc = _build()
    in_maps = _host_prep(**inputs)
    res = run_bass_kernel_spmd(nc, in_maps, list(range(NCORES)))
    out = np.empty((B, 1), np.float32)
    for i in range(NCORES):
        out[i * BLOC:(i + 1) * BLOC, 0] = res.results[i]["y"][0]
    return out


# revision 18
# speedup vs baseline: 1.0667x; 1.0667x over previous
"""Trainium2 Bass kernel for ChebConv(K=2) + MLP net (nn_Net_63299228009240).

Data-parallel over 8 NeuronCores: batch 4096 -> 512 per core, one pass
(NB=512) per core.

Math (per sample, reordered to cut SpMM flops):
    u = x @ w0; v = x @ w1            [200, 32]
    t1 = a @ v                        ((a @ x) @ w1 == a @ (x @ w1))
    h = elu(u + t1 + b_conv)          -> flat [6400]
    y1 = relu(h @ wd1 + bd1); y2 = relu(y1 @ wd2 + bd2)
    y3 = relu(y2 @ wd3 + bd3); out = sigmoid(y3 @ wd4 + bd4)

elu(s) = relu(s) + exp(min(s, 0)) - 1; the -1 is folded into bd1 on host.

Layout: activations are "feature-on-partition, batch-on-free". x comes
pre-transposed per core as XT[(n,f), b] node-pair tiles. U bounces
through DRAM to free SBUF; wd1 streams from HBM during d1. The SpMM
needs node-on-partition, so V is shuffled [(n%4,c),b] -> [n,(c,b)] via
one SBUF->SBUF DMA per conv psum tile (the (q,c) partition walk is
monotone), and T1 lands back in k-tile layout with one DMA per (mt,c)
psum tile thanks to an r-major host permutation of at's columns.
"""

import sys

sys.path.insert(0, "/opt/trn_rl_repo")

from contextlib import ExitStack

import numpy as np
import ml_dtypes

import concourse.mybir as mybir
import concourse.tile as tile
from concourse import bacc
from concourse.alu_op_type import AluOpType
from concourse.bass_utils import run_bass_kernel_spmd

BF16 = mybir.dt.bfloat16
F32 = mybir.dt.float32
NPBF16 = ml_dtypes.bfloat16

NCORES = 8
B, NN, F, C = 4096, 200, 64, 32
H1, H2, H3 = 512, 256, 128
BLOC = B // NCORES  # 512 samples per core
NB = 512            # full per-core batch in one pass
NT = NN // 2        # 100 node-pair tiles [(2 nodes x 64 f) = 128, NB]
KT = NN // 4        # 50 H k-tiles [(4 nodes x 32 c) = 128, NB]
XBLK = 10           # node-pairs per XT load DMA
MT1 = 100           # SpMM output tile rows (n_out per tile)

_CACHE = {}


def _emit(nc, tc, ctx, d):
    singles = ctx.enter_context(tc.tile_pool(name="singles", bufs=1))
    xtp = ctx.enter_context(tc.tile_pool(name="xtp", bufs=2))
    wd1p = ctx.enter_context(tc.tile_pool(name="wd1p", bufs=2))
    udp = ctx.enter_context(tc.tile_pool(name="udp", bufs=2))
    uevp = ctx.enter_context(tc.tile_pool(name="uevp", bufs=4))
    t1kp = ctx.enter_context(tc.tile_pool(name="t1kp", bufs=1))
    vpool = ctx.enter_context(tc.tile_pool(name="vpool", bufs=1))
    vstp = ctx.enter_context(tc.tile_pool(name="vstp", bufs=3))
    t1stp = ctx.enter_context(tc.tile_pool(name="t1stp", bufs=4))
    sp = ctx.enter_context(tc.tile_pool(name="sp", bufs=3))
    mnp = ctx.enter_context(tc.tile_pool(name="mnp", bufs=3))
    ep = ctx.enter_context(tc.tile_pool(name="ep", bufs=3))
    gp = ctx.enter_context(tc.tile_pool(name="gp", bufs=4))
    yp = ctx.enter_context(tc.tile_pool(name="yp", bufs=1))
    drp = ctx.enter_context(tc.tile_pool(name="drp", bufs=1, space="DRAM"))
    ps = ctx.enter_context(tc.tile_pool(name="ps", bufs=4, space="PSUM"))
    psC = ctx.enter_context(tc.tile_pool(name="psC", bufs=4, space="PSUM"))

    # ---- load weights/constants once ----
    wua = singles.tile([128, 128], BF16)
    wub = singles.tile([128, 128], BF16)
    wva = singles.tile([128, 128], BF16)
    wvb = singles.tile([128, 128], BF16)
    atp = singles.tile([128, 2, NN], BF16)
    wd2 = singles.tile([128, 4, H2], BF16)
    wd3 = singles.tile([128, 2, H3], BF16)
    wd4 = singles.tile([128, 1], BF16)
    bcv = singles.tile([128, 1], F32)
    bd1 = singles.tile([128, 4], F32)
    bd2 = singles.tile([128, 2], F32)
    bd3 = singles.tile([128, 1], F32)
    bd4 = singles.tile([1, 1], F32)
    for sb, nm in [(wua, "wua"), (wub, "wub"), (wva, "wva"), (wvb, "wvb"),
                   (atp, "atp"), (wd2, "wd2s"), (wd3, "wd3s"),
                   (wd4, "wd4s"), (bcv, "bconv"), (bd1, "bd1"),
                   (bd2, "bd2"), (bd3, "bd3"), (bd4, "bd4")]:
        nc.sync.dma_start(out=sb[:], in_=d[nm][:])

    # ---- XT loads (double-buffered blocks of XBLK node-pairs) ----
    xtb = []
    for xb in range(NT // XBLK):
        t = xtp.tile([128, XBLK, NB], BF16, tag="xt", name=f"xt{xb}")
        nc.sync.dma_start(
            out=t[:], in_=d["xt"][:, 0, xb * XBLK:(xb + 1) * XBLK, :])
        xtb.append(t)

    def xtile(t):
        return xtb[t // XBLK][:, t % XBLK, :]

    t1k = t1kp.tile([128, KT, NB], BF16, tag="t1k")
    v0 = vpool.tile([128, 32, NB], BF16, tag="v0")
    v1 = vpool.tile([128, 32, NB], BF16, tag="v1")
    ud = drp.tile([128, KT, NB], BF16, tag="ud")  # U bounce (DRAM)

    # ---- conv matmuls + evac; V shuffled to [n,(c,b)], U bounced out ----
    for pt in range(KT):  # psum tile pt covers nodes 4pt..4pt+3
        t0, t1_ = 2 * pt, 2 * pt + 1
        up = ps.tile([128, NB], F32, tag="ps", name=f"up{pt}")
        nc.tensor.matmul(up[:], wua[:], xtile(t0), start=True, stop=False)
        nc.tensor.matmul(up[:], wub[:], xtile(t1_), start=False, stop=True)
        # U evac with conv bias folded in (k-tile layout [(n%4,c), b])
        uev = uevp.tile([128, NB], BF16, tag="uev", name=f"uev{pt}")
        nc.vector.tensor_scalar_add(uev[:], up[:], bcv[:])
        nc.sync.dma_start(out=ud[:, pt, :], in_=uev[:])

        vps = ps.tile([128, NB], F32, tag="ps", name=f"vp{pt}")
        nc.tensor.matmul(vps[:], wva[:], xtile(t0), start=True, stop=False)
        nc.tensor.matmul(vps[:], wvb[:], xtile(t1_), start=False, stop=True)
        vst = vstp.tile([128, NB], BF16, tag="vst", name=f"vst{pt}")
        nc.scalar.copy(out=vst[:], in_=vps[:])
        # V shuffle: [(n%4)*32+c, b] -> V[4pt+q, c, b]; src partition walk
        # (q, c) is monotone so one DMA moves all 4 nodes.
        if pt < 32:
            dst = v0[4 * pt:4 * pt + 4, :, :]
        else:
            dst = v1[4 * (pt - 32):4 * (pt - 32) + 4, :, :]
        nc.sync.dma_start(out=dst, in_=vst[:])

    # ---- SpMM: T1[n_out, b] = aT.T @ V, one psum tile per (mt, c) ----
    # at columns are host-permuted r-major (m = r*25 + q for n_out =
    # mt*100 + 4q + r), so each psum tile's partition walk matches
    # t1k[r*32+c, 25mt+q, b] directly: one SBUF->SBUF DMA per tile.
    t1kv = t1k.rearrange("(r c) k b -> c r k b", r=4)
    for mt in range(2):
        lhs0 = atp[:, 0, mt * MT1:(mt + 1) * MT1]
        lhs1 = atp[0:72, 1, mt * MT1:(mt + 1) * MT1]
        for c in range(C):
            bp = ps.tile([MT1, NB], F32, tag="ps", name=f"bp{mt}_{c}")
            nc.tensor.matmul(bp[:], lhs0, v0[:, c, :], start=True, stop=False)
            nc.tensor.matmul(bp[:], lhs1, v1[0:72, c, :], start=False,
                             stop=True)
            t1s = t1stp.tile([MT1, NB], BF16, tag="t1st", name=f"t1s{mt}_{c}")
            nc.vector.tensor_copy(t1s[:], bp[:])
            nc.scalar.dma_start(
                out=t1kv[c, :, 25 * mt:25 * (mt + 1), :], in_=t1s[:])

    # ---- U reload + ELU -> G, d1 matmuls (kt-outer, 4 open groups) ----
    # wd1 streams from HBM; U streams back from its DRAM bounce.
    wd1b = []
    for j in range(5):
        w = wd1p.tile([128, XBLK, H1], BF16, tag="wd1", name=f"wd1b{j}")
        nc.sync.dma_start(out=w[:], in_=d["wd1s"][:, j * XBLK:(j + 1) * XBLK, :])
        wd1b.append(w)
    udb = []
    for j in range(10):
        u = udp.tile([128, 5, NB], BF16, tag="ud", name=f"udb{j}")
        nc.sync.dma_start(out=u[:], in_=ud[:, j * 5:(j + 1) * 5, :])
        udb.append(u)

    d1c = [psC.tile([128, NB], F32, tag="psC", name=f"d1c{j}")
           for j in range(4)]
    for kt in range(KT):
        s = sp.tile([128, NB], BF16, tag="s", name=f"s{kt}")
        nc.vector.tensor_add(s[:], udb[kt // 5][:, kt % 5, :], t1k[:, kt, :])
        mn = mnp.tile([128, NB], BF16, tag="mn", name=f"mn{kt}")
        nc.vector.tensor_scalar_min(mn[:], s[:], 0.0)
        e = ep.tile([128, NB], BF16, tag="e", name=f"e{kt}")
        nc.scalar.activation(e[:], mn[:], mybir.ActivationFunctionType.Exp)
        g = gp.tile([128, NB], BF16, tag="g", name=f"g{kt}")
        nc.vector.scalar_tensor_tensor(
            out=g[:], in0=s[:], scalar=0.0, in1=e[:],
            op0=AluOpType.max, op1=AluOpType.add)
        for mt in range(4):
            nc.tensor.matmul(
                d1c[mt][:], wd1b[kt // XBLK][:, kt % XBLK, mt * 128:(mt + 1) * 128],
                g[:], start=(kt == 0), stop=(kt == KT - 1))

    y1 = yp.tile([128, 4, NB], BF16, tag="y1")
    for mt in range(4):
        nc.scalar.activation(
            y1[:, mt, :], d1c[mt][:],
            mybir.ActivationFunctionType.Relu, bias=bd1[:, mt:mt + 1])

    # ---- d2 ----
    y2 = yp.tile([128, 2, NB], BF16, tag="y2")
    for mt in range(2):
        dp = ps.tile([128, NB], F32, tag="ps", name=f"d2p{mt}")
        for kt in range(4):
            nc.tensor.matmul(
                dp[:], wd2[:, kt, mt * 128:(mt + 1) * 128], y1[:, kt, :],
                start=(kt == 0), stop=(kt == 3))
        nc.scalar.activation(
            y2[:, mt, :], dp[:],
            mybir.ActivationFunctionType.Relu, bias=bd2[:, mt:mt + 1])

    # ---- d3 ----
    y3 = yp.tile([128, NB], BF16, tag="y3")
    dp3 = ps.tile([128, NB], F32, tag="ps", name="d3p")
    for kt in range(2):
        nc.tensor.matmul(dp3[:], wd3[:, kt, :], y2[:, kt, :],
                         start=(kt == 0), stop=(kt == 1))
    nc.scalar.activation(y3[:], dp3[:],
                         mybir.ActivationFunctionType.Relu, bias=bd3[:])

    # ---- d4 + sigmoid ----
    dp4 = ps.tile([1, NB], F32, tag="ps", name="d4p")
    nc.tensor.matmul(dp4[:], wd4[:], y3[:], start=True, stop=True)
    yo = yp.tile([1, NB], F32, tag="yo")
    nc.scalar.activation(yo[:], dp4[:],
                         mybir.ActivationFunctionType.Sigmoid, bias=bd4[:])
    nc.sync.dma_start(out=d["y"][0:1, :], in_=yo[:])


def _build():
    if "nc" in _CACHE:
        return _CACHE["nc"]
    nc = bacc.Bacc("TRN2", target_bir_lowering=False, debug=False,
                   num_devices=NCORES)
    d = {
        "xt": nc.dram_tensor("xt", [128, 1, NT, NB], BF16, kind="ExternalInput"),
        "wua": nc.dram_tensor("wua", [128, 128], BF16, kind="ExternalInput"),
        "wub": nc.dram_tensor("wub", [128, 128], BF16, kind="ExternalInput"),
        "wva": nc.dram_tensor("wva", [128, 128], BF16, kind="ExternalInput"),
        "wvb": nc.dram_tensor("wvb", [128, 128], BF16, kind="ExternalInput"),
        "atp": nc.dram_tensor("atp", [128, 2, NN], BF16, kind="ExternalInput"),
        "wd1s": nc.dram_tensor("wd1s", [128, KT, H1], BF16, kind="ExternalInput"),
        "wd2s": nc.dram_tensor("wd2s", [128, 4, H2], BF16, kind="ExternalInput"),
        "wd3s": nc.dram_tensor("wd3s", [128, 2, H3], BF16, kind="ExternalInput"),
        "wd4s": nc.dram_tensor("wd4s", [128, 1], BF16, kind="ExternalInput"),
        "bconv": nc.dram_tensor("bconv", [128, 1], F32, kind="ExternalInput"),
        "bd1": nc.dram_tensor("bd1", [128, 4], F32, kind="ExternalInput"),
        "bd2": nc.dram_tensor("bd2", [128, 2], F32, kind="ExternalInput"),
        "bd3": nc.dram_tensor("bd3", [128, 1], F32, kind="ExternalInput"),
        "bd4": nc.dram_tensor("bd4", [1, 1], F32, kind="ExternalInput"),
        "y": nc.dram_tensor("y", [1, BLOC], F32, kind="ExternalOutput"),
    }
    with tile.TileContext(nc) as tc, ExitStack() as ctx:
        _emit(nc, tc, ctx, d)
    nc.compile()
    _CACHE["nc"] = nc
    return nc


def _host_prep(x, a, w0, w1, b_conv, w_d1, b_d1, w_d2, b_d2, w_d3, b_d3,
               w_d4, b_d4):
    """Build per-core in_maps (shared weight arrays built once)."""
    w0 = np.asarray(w0, np.float32)
    w1 = np.asarray(w1, np.float32)
    # block-diag conv weights: lhsT[(nl*64+f), m]; A-variant fills out cols
    # [0:64) (nodes 4pt,4pt+1), B-variant cols [64:128) (nodes 4pt+2,4pt+3)
    wua = np.zeros((128, 128), np.float32)
    wub = np.zeros((128, 128), np.float32)
    wva = np.zeros((128, 128), np.float32)
    wvb = np.zeros((128, 128), np.float32)
    for nl in range(2):
        r = slice(nl * 64, nl * 64 + 64)
        wua[r, nl * 32:nl * 32 + 32] = w0
        wub[r, 64 + nl * 32:64 + nl * 32 + 32] = w0
        wva[r, nl * 32:nl * 32 + 32] = w1
        wvb[r, 64 + nl * 32:64 + nl * 32 + 32] = w1

    # permute at columns r-major within each mt block: col mt*100+r*25+q
    # holds n_out = mt*100+4q+r, so SpMM psum partitions land t1k-ready
    perm = np.empty(NN, np.int64)
    for mt in range(2):
        for q in range(25):
            for r in range(4):
                perm[mt * 100 + r * 25 + q] = mt * 100 + 4 * q + r
    atp = np.zeros((128, 2, NN), np.float32)
    at = np.asarray(a, np.float32).T[:, perm]  # at[n, m]
    atp[:, 0, :] = at[0:128, :]
    atp[0:72, 1, :] = at[128:200, :]

    w_d1 = np.asarray(w_d1, np.float32)
    wd1s = np.ascontiguousarray(w_d1.reshape(KT, 128, H1).transpose(1, 0, 2))
    wd2s = np.ascontiguousarray(
        np.asarray(w_d2, np.float32).reshape(4, 128, H2).transpose(1, 0, 2))
    wd3s = np.ascontiguousarray(
        np.asarray(w_d3, np.float32).reshape(2, 128, H3).transpose(1, 0, 2))
    wd4s = np.asarray(w_d4, np.float32).reshape(128, 1)

    p = np.arange(128)
    bconv_v = np.asarray(b_conv, np.float32)[p % 32].reshape(128, 1)
    bd1c = np.asarray(b_d1, np.float32) - w_d1.sum(axis=0)  # fold elu's -1
    bd1_v = np.ascontiguousarray(bd1c.reshape(4, 128).T)
    bd2_v = np.ascontiguousarray(
        np.asarray(b_d2, np.float32).reshape(2, 128).T)
    bd3_v = np.asarray(b_d3, np.float32).reshape(128, 1)
    bd4_v = np.asarray(b_d4, np.float32).reshape(1, 1)

    shared = {
        "wua": wua.astype(NPBF16), "wub": wub.astype(NPBF16),
        "wva": wva.astype(NPBF16), "wvb": wvb.astype(NPBF16),
        "atp": atp.astype(NPBF16), "wd1s": wd1s.astype(NPBF16),
        "wd2s": wd2s.astype(NPBF16), "wd3s": wd3s.astype(NPBF16),
        "wd4s": wd4s.astype(NPBF16),
        "bconv": np.ascontiguousarray(bconv_v, np.float32),
        "bd1": bd1_v.astype(np.float32), "bd2": bd2_v.astype(np.float32),
        "bd3": bd3_v, "bd4": bd4_v,
    }

    x = np.asarray(x, np.float32)
    in_maps = []
    for i in range(NCORES):
        xs = x[i * BLOC:(i + 1) * BLOC]                    # [512, 200, 64]
        xtf = xs.transpose(1, 2, 0).reshape(NN * F, BLOC)  # [(n,f), b]
        xti = np.ascontiguousarray(
            xtf.reshape(NT, 128, 1, NB).transpose(1, 2, 0, 3)).astype(NPBF16)
        in_maps.append({"xt": xti, **shared})
    return in_maps


def kernel(**inputs):
    nc = _build()
    in_maps = _host_prep(**inputs)
    res = run_bass_kernel_spmd(nc, in_maps, list(range(NCORES)))
    out = np.empty((B, 1), np.float32)
    for i in range(NCORES):
        out[i * BLOC:(i + 1) * BLOC, 0] = res.results[i]["y"][0]
    return out
